# revision 13
# baseline (speedup 1.0000x reference)
"""Cellsort Hamiltonian on 8 Trainium2 NeuronCores.

Computation (see reference):
  ham = (softplus(lamb)+1e-3) * sum_{id=1..199}(bincount(ids)[id] - v_pref)^2
        + (1/4) * sum_{4 offsets} sum_pixels [id != id_nbr] * J_eff[t, t_nbr]
        + offset*offset_scale

Device strategy (SPMD over 8 cores, row-sharded 512 rows/core + 1 halo row):
  - 200-bin histogram split across two engines:
      * DVE: tensor_scalar(is_equal)+accum_out passes (int16, 4x mode) over a
        full-width ids tile (free dim 16384 amortizes per-instr overhead)
      * ACT: Sign-CDF trick -- S(b) = sum sign(x-b+0.5) accumulated per
        threshold; n_b = (S(b)-S(b+1))/2 recovered on the host
  - interaction: per offset build ckey = 3*t + t_nbr + 9*[id==id_nbr] on DVE,
    collect ckey for offset-pairs into a shared tile, count bins 0..8 (the
    [id!=id_nbr] pair-type counts, symmetric J makes scaled-side choice free).
  Device outputs integer counts / sign-sums (as f32); host does all float math.

Layout per core: rows split into 4 blocks of 128 partitions. ids live in one
full-width tile [128, 4, 4100] (payload cols 2..4097, one wrap col each side).
Type and row-below tiles are column quarters [128, 4, 1026] (1024 payload + 2
wrap cols) cut from a host-padded [513, 4098] input, so every stencil neighbor
(j wrap and halo row included) is a pure AP shift.
"""

import numpy as np

import concourse.bacc as bacc
import concourse.mybir as mybir
from concourse.tile import TileContext
from concourse.bass_utils import run_bass_kernel_spmd

H = W = 4096
NCORES = 8
ROWS = H // NCORES          # 512 rows per core
NBLK = ROWS // 128          # 4 partition blocks
NQ = 4                      # column quarters
QCOL = W // NQ              # 1024 payload cols per quarter
NBINS = 200
NPAIR = 9                   # 3x3 type-pair bins

DVE_BINS = 137              # bins 1..DVE_BINS on DVE; rest via ACT sign-CDF

OFFSETS = [(0, 1), (1, 0), (1, 1), (1, -1)]

_CACHE = {}


def _build(dve_bins=DVE_BINS):
    # DVE counts bins 1..dve_bins; ACT sign-CDF covers dve_bins+1..199.
    # Bin 0 is never needed (vol_term sums bins 1..199).
    act_thr = NBINS - 1 - dve_bins
    nc = bacc.Bacc("TRN2", debug=False)
    i32, i16, f32 = mybir.dt.int32, mybir.dt.int16, mybir.dt.float32
    A = mybir.AluOpType
    Sign = mybir.ActivationFunctionType.Sign

    ids_d = nc.dram_tensor("ids", [ROWS + 1, W + 2], i16, kind="ExternalInput")
    typ_d = nc.dram_tensor("typ", [ROWS + 1, W + 2], i16, kind="ExternalInput")
    thr_d = nc.dram_tensor("thr", [1, max(act_thr, 1)], f32, kind="ExternalInput")
    hist_d = nc.dram_tensor("hist_out", [1, dve_bins], f32, kind="ExternalOutput")
    sgn_d = nc.dram_tensor("sgn_out", [1, max(act_thr, 1)], f32, kind="ExternalOutput")
    icnt_d = nc.dram_tensor("icnt_out", [1, NPAIR], f32, kind="ExternalOutput")

    # DRAM views: row r = 128*b + p  ->  [p, b, c]
    ids_top = ids_d[0:ROWS, :].rearrange("(b p) c -> p b c", p=128)
    typ_top = typ_d[0:ROWS, :].rearrange("(b p) c -> p b c", p=128)

    with TileContext(nc) as tc:
        with (
            tc.tile_pool(name="io", bufs=2) as io_pool,
            tc.tile_pool(name="big", bufs=1) as big_pool,
            tc.tile_pool(name="scratch", bufs=1) as s_pool,
            tc.tile_pool(name="acc", bufs=1) as acc_pool,
            tc.tile_pool(name="psum", bufs=1, space="PSUM") as psum_pool,
        ):
            counts = acc_pool.tile([128, dve_bins], f32, tag="counts")
            sgns = acc_pool.tile([128, max(act_thr, 1)], f32, tag="sgns")
            icnts = acc_pool.tile([128, NQ * 2 * NPAIR], f32, tag="icnts")
            ones = acc_pool.tile([128, 1], f32, tag="ones")
            nc.vector.memset(ones[:], 1.0)
            thr = acc_pool.tile([128, max(act_thr, 1)], f32, tag="thr")
            nc.sync.dma_start(out=thr[:], in_=thr_d[:, :].partition_broadcast(128))

            # full-width ids tile: col k holds image col k-2 (k=1..4098 loaded)
            idsF = big_pool.tile([128, NBLK, W + 4], i16, tag="idsF")
            nc.sync.dma_start(out=idsF[:, :, 1 : W + 3], in_=ids_top[:, :, :])

            # --- histogram, DVE part: full-width passes ---
            ids_all = idsF[:, :, 2 : W + 2]
            junk = s_pool.tile([128, NBLK, W], i16, tag="dscratch")
            for b in range(1, dve_bins + 1):
                nc.vector.tensor_scalar(
                    out=junk[:],
                    in0=ids_all,
                    scalar1=float(b),
                    scalar2=None,
                    op0=A.is_equal,
                    op1=A.add,
                    accum_out=counts[:, b - 1 : b],
                )

            # --- histogram, ACT sign-CDF part: full-width passes ---
            junk_a = s_pool.tile([128, NBLK, W], i16, tag="junk_a")
            for j in range(act_thr):
                nc.scalar.activation(
                    out=junk_a[:],
                    in_=ids_all,
                    func=Sign,
                    bias=thr[:, j : j + 1],
                    scale=1.0,
                    accum_out=sgns[:, j : j + 1],
                )

            # ckey fields for two offsets at a time
            ck2 = big_pool.tile([128, 2 * NBLK, QCOL], i16, tag="ck2")

            # halo row (image row 512*m+512) for the down-shifted tiles
            hal_i = acc_pool.tile([1, W + 2], i16, tag="hal_i")
            hal_t = acc_pool.tile([1, W + 2], i16, tag="hal_t")
            nc.sync.dma_start(out=hal_i[:], in_=ids_d[ROWS : ROWS + 1, :])
            nc.sync.dma_start(out=hal_t[:], in_=typ_d[ROWS : ROWS + 1, :])

            for q in range(NQ):
                c0 = q * QCOL  # strip covers padded cols [c0, c0+1026)
                sl = slice(c0, c0 + QCOL + 2)

                typ = io_pool.tile([128, NBLK, QCOL + 2], i16, tag="typ")
                idn = io_pool.tile([128, NBLK, QCOL + 2], i16, tag="idn")
                tdn = io_pool.tile([128, NBLK, QCOL + 2], i16, tag="tdn")
                t3 = io_pool.tile([128, NBLK, QCOL + 2], i16, tag="t3")

                nc.sync.dma_start(out=typ[:], in_=typ_top[:, :, sl])
                # row-below tiles built on-chip: partition shift within SBUF
                fsl = slice(c0 + 1, c0 + 1 + QCOL + 2)  # same strip in idsF cols
                nc.sync.dma_start(out=idn[0:127, :, :], in_=idsF[1:128, :, fsl])
                nc.sync.dma_start(
                    out=idn[127:128, 0 : NBLK - 1, :], in_=idsF[0:1, 1:NBLK, fsl]
                )
                nc.sync.dma_start(
                    out=idn[127:128, NBLK - 1, :], in_=hal_i[:, sl]
                )
                nc.sync.dma_start(out=tdn[0:127, :, :], in_=typ[1:128, :, :])
                nc.sync.dma_start(
                    out=tdn[127:128, 0 : NBLK - 1, :], in_=typ[0:1, 1:NBLK, :]
                )
                nc.sync.dma_start(
                    out=tdn[127:128, NBLK - 1, :], in_=hal_t[:, sl]
                )

                # t3 = 3*typ on the ACT engine
                nc.scalar.mul(t3[:], typ[:], 3.0)

                # self views (payload cols of this quarter)
                ids_s = idsF[:, :, 2 + c0 : 2 + c0 + QCOL]
                t3_s = t3[:, :, 1 : QCOL + 1]

                # --- interaction ckey fields, two offsets per count group ---
                for g in range(2):
                    for oo in range(2):
                        o = 2 * g + oo
                        di, dj = OFFSETS[o]
                        if di == 0:
                            ids_n = idsF[:, :, 2 + c0 + dj : 2 + c0 + dj + QCOL]
                            t_n = typ[:, :, 1 + dj : QCOL + 1 + dj]
                        else:
                            ids_n = idn[:, :, 1 + dj : QCOL + 1 + dj]
                            t_n = tdn[:, :, 1 + dj : QCOL + 1 + dj]

                        s_eq = s_pool.tile([128, NBLK, QCOL], i16, tag="dscratch")
                        s_e9 = s_pool.tile([128, NBLK, QCOL], i16, tag="s_e9")
                        s_ky = s_pool.tile([128, NBLK, QCOL], i16, tag="dscratch")

                        nc.vector.tensor_tensor(
                            out=s_eq[:], in0=ids_s, in1=ids_n, op=A.is_equal
                        )
                        nc.vector.tensor_scalar(
                            out=s_e9[:], in0=s_eq[:], scalar1=9.0, scalar2=None,
                            op0=A.mult,
                        )
                        nc.vector.tensor_tensor(
                            out=s_ky[:], in0=t3_s, in1=t_n, op=A.add
                        )
                        nc.vector.tensor_tensor(
                            out=ck2[:, oo * NBLK : (oo + 1) * NBLK, :],
                            in0=s_ky[:],
                            in1=s_e9[:],
                            op=A.add,
                        )
                    # count 9 pair bins over this 2-offset group
                    junk_c = s_pool.tile([128, 2 * NBLK, QCOL], i16, tag="dscratch")
                    for v in range(NPAIR):
                        col = (q * 2 + g) * NPAIR + v
                        nc.vector.tensor_scalar(
                            out=junk_c[:],
                            in0=ck2[:],
                            scalar1=float(v),
                            scalar2=None,
                            op0=A.is_equal,
                            op1=A.add,
                            accum_out=icnts[:, col : col + 1],
                        )

            # --- reduce partials across partitions with PE ones-matmul ---
            def pe_reduce(src, dst_dram, width):
                sb = acc_pool.tile([1, width], f32, tag=f"sb_{dst_dram.name}")
                for lo in range(0, width, 400):
                    hi = min(lo + 400, width)
                    ps = psum_pool.tile(
                        [1, 400], f32, tag=f"ps_{dst_dram.name}_{lo}", space="PSUM"
                    )
                    nc.tensor.matmul(
                        ps[:, : hi - lo], ones[:], src[:, lo:hi], start=True, stop=True
                    )
                    nc.scalar.copy(sb[:, lo:hi], ps[:, : hi - lo])
                nc.sync.dma_start(out=dst_dram[:, :], in_=sb[:])

            pe_reduce(counts, hist_d, dve_bins)
            pe_reduce(sgns, sgn_d, max(act_thr, 1))

            icnt_sum = acc_pool.tile([128, NPAIR], f32, tag="icnt_sum")
            # fold the NQ*2 groups: view [128, NQ*2, NPAIR] -> reduce groups on DVE
            nc.vector.tensor_reduce(
                out=icnt_sum[:],
                in_=icnts[:].rearrange("p (g v) -> p v g", v=NPAIR),
                op=A.add,
                axis=mybir.AxisListType.X,
            )
            pe_reduce(icnt_sum, icnt_d, NPAIR)

    nc.finalize()
    return nc


def _get_nc():
    if "nc" not in _CACHE:
        _CACHE["nc"] = _build()
    return _CACHE["nc"]


def _softplus(x):
    x = np.asarray(x, np.float64)
    return np.log1p(np.exp(-np.abs(x))) + np.maximum(x, 0.0)


def _make_in_maps(cell_ids, cell_types, dve_bins=DVE_BINS):
    ids = np.ascontiguousarray(cell_ids, dtype=np.int16)
    typ = np.ascontiguousarray(cell_types, dtype=np.int16)
    act_thr = NBINS - 1 - dve_bins
    if act_thr:
        thr = (0.5 - np.arange(dve_bins + 1, NBINS, dtype=np.float64)).astype(np.float32)
        thr = np.ascontiguousarray(thr.reshape(1, -1))
    else:
        thr = np.zeros((1, 1), np.float32)

    def shard(x, m):
        rows = np.arange(m * ROWS, m * ROWS + ROWS + 1) % H
        s = x[rows]  # [513, 4096]
        return np.ascontiguousarray(
            np.concatenate([s[:, -1:], s, s[:, :1]], axis=1)
        )  # [513, 4098]

    return [
        {"ids": shard(ids, m), "typ": shard(typ, m), "thr": thr}
        for m in range(NCORES)
    ]


def kernel(
    cell_ids, cell_types, J, gamma_J, bias_J, v_pref, lamb, offset, offset_scale
):
    nc = _get_nc()
    in_maps = _make_in_maps(cell_ids, cell_types)
    res = run_bass_kernel_spmd(nc, in_maps, core_ids=list(range(NCORES)))

    act_thr = NBINS - 1 - DVE_BINS
    hist = np.zeros(NBINS, np.float64)
    pair = np.zeros(NPAIR, np.float64)
    qpix = float(128 * NBLK * QCOL)  # pixels per quarter
    for r in res.results:
        hist[1 : DVE_BINS + 1] += r["hist_out"].reshape(DVE_BINS).astype(np.float64)
        if act_thr:
            S = r["sgn_out"].reshape(act_thr).astype(np.float64)  # S(b0+1..199)
            Sn = np.concatenate([S, [-4.0 * qpix]])  # append S(200)
            hist[DVE_BINS + 1 :] += (Sn[:-1] - Sn[1:]) / 2.0
        pair += r["icnt_out"].reshape(NPAIR).astype(np.float64)

    # symmetrize: ckey used 3*t_self + t_nbr with J symmetric
    J_eff = (
        _softplus(np.float64(gamma_J[0])) * np.asarray(J, np.float64)
        + np.float64(bias_J[0])
    )
    inter = float((J_eff.reshape(-1) * pair).sum()) / len(OFFSETS)
    vol = float(
        ((hist[1:] - np.float64(v_pref[0])) ** 2).sum()
        * (_softplus(np.float64(lamb[0])) + 0.001)
    )
    ham = vol + inter + float(offset[0]) * float(offset_scale[0])
    return np.array([ham], dtype=np.float32)


# revision 16
# speedup vs baseline: 1.0254x; 1.0254x over previous
"""Cellsort Hamiltonian on 8 Trainium2 NeuronCores.

Computation (see reference):
  ham = (softplus(lamb)+1e-3) * sum_{id=1..199}(bincount(ids)[id] - v_pref)^2
        + (1/4) * sum_{4 offsets} sum_pixels [id != id_nbr] * J_eff[t, t_nbr]
        + offset*offset_scale

Device strategy (SPMD over 8 cores, row-sharded 512 rows/core + 1 halo row):
  - 200-bin histogram split across two engines:
      * DVE: tensor_scalar(is_equal)+accum_out passes (int16, 4x mode) over a
        full-width ids tile (free dim 16384 amortizes per-instr overhead)
      * ACT: Sign-CDF trick -- S(b) = sum sign(x-b+0.5) accumulated per
        threshold; n_b = (S(b)-S(b+1))/2 recovered on the host
  - interaction: per offset build ckey = 3*t + t_nbr + 9*[id==id_nbr] on DVE,
    collect ckey for offset-pairs into a shared tile, count bins 0..8 (the
    [id!=id_nbr] pair-type counts, symmetric J makes scaled-side choice free).
  Device outputs integer counts / sign-sums (as f32); host does all float math.

Layout per core: rows split into 4 blocks of 128 partitions. ids live in one
full-width tile [128, 4, 4100] (payload cols 2..4097, one wrap col each side).
Type and row-below tiles are column quarters [128, 4, 1026] (1024 payload + 2
wrap cols) cut from a host-padded [513, 4098] input, so every stencil neighbor
(j wrap and halo row included) is a pure AP shift.
"""

import numpy as np

import concourse.bacc as bacc
import concourse.mybir as mybir
from concourse.tile import TileContext
from concourse.bass_utils import run_bass_kernel_spmd

H = W = 4096
NCORES = 8
ROWS = H // NCORES          # 512 rows per core
NBLK = ROWS // 128          # 4 partition blocks
NQ = 4                      # column quarters
QCOL = W // NQ              # 1024 payload cols per quarter
NBINS = 200
NPAIR = 9                   # 3x3 type-pair bins

DVE_BINS = 138              # bins 1..DVE_BINS on DVE; rest via ACT sign-CDF

OFFSETS = [(0, 1), (1, 0), (1, 1), (1, -1)]

_CACHE = {}


def _build(dve_bins=DVE_BINS):
    # DVE counts bins 1..dve_bins; ACT sign-CDF covers dve_bins+1..199.
    # Bin 0 is never needed (vol_term sums bins 1..199).
    act_thr = NBINS - 1 - dve_bins
    nc = bacc.Bacc("TRN2", debug=False)
    i32, i16, f32 = mybir.dt.int32, mybir.dt.int16, mybir.dt.float32
    A = mybir.AluOpType
    Sign = mybir.ActivationFunctionType.Sign

    ids_d = nc.dram_tensor("ids", [ROWS + 1, W + 2], i16, kind="ExternalInput")
    typ_d = nc.dram_tensor("typ", [ROWS + 1, W + 2], i16, kind="ExternalInput")
    thr_d = nc.dram_tensor("thr", [1, max(act_thr, 1)], f32, kind="ExternalInput")
    hist_d = nc.dram_tensor("hist_out", [1, dve_bins], f32, kind="ExternalOutput")
    sgn_d = nc.dram_tensor("sgn_out", [1, max(act_thr, 1)], f32, kind="ExternalOutput")
    icnt_d = nc.dram_tensor("icnt_out", [1, NPAIR], f32, kind="ExternalOutput")

    # DRAM views: row r = 128*b + p  ->  [p, b, c]
    ids_top = ids_d[0:ROWS, :].rearrange("(b p) c -> p b c", p=128)
    typ_top = typ_d[0:ROWS, :].rearrange("(b p) c -> p b c", p=128)

    with TileContext(nc) as tc:
        with (
            tc.tile_pool(name="io", bufs=2) as io_pool,
            tc.tile_pool(name="big", bufs=1) as big_pool,
            tc.tile_pool(name="scratch", bufs=1) as s_pool,
            tc.tile_pool(name="acc", bufs=1) as acc_pool,
            tc.tile_pool(name="psum", bufs=1, space="PSUM") as psum_pool,
        ):
            counts = acc_pool.tile([128, dve_bins], f32, tag="counts")
            sgns = acc_pool.tile([128, max(act_thr, 1)], f32, tag="sgns")
            icnts = acc_pool.tile([128, NQ * NPAIR], f32, tag="icnts")
            ones = acc_pool.tile([128, 1], f32, tag="ones")
            nc.vector.memset(ones[:], 1.0)
            thr = acc_pool.tile([128, max(act_thr, 1)], f32, tag="thr")
            nc.sync.dma_start(out=thr[:], in_=thr_d[:, :].partition_broadcast(128))

            # full-width ids tile: col k holds image col k-2 (k=1..4098 loaded)
            idsF = big_pool.tile([128, NBLK, W + 4], i16, tag="idsF")
            nc.sync.dma_start(out=idsF[:, :, 1 : W + 3], in_=ids_top[:, :, :])

            # --- histogram, DVE part: full-width passes ---
            ids_all = idsF[:, :, 2 : W + 2]
            junk = s_pool.tile([128, NBLK, W], i16, tag="dscratch")
            for b in range(1, dve_bins + 1):
                nc.vector.tensor_scalar(
                    out=junk[:],
                    in0=ids_all,
                    scalar1=float(b),
                    scalar2=None,
                    op0=A.is_equal,
                    op1=A.add,
                    accum_out=counts[:, b - 1 : b],
                )

            # --- histogram, ACT sign-CDF part: full-width passes ---
            junk_a = s_pool.tile([128, NBLK, W], i16, tag="junk_a")
            for j in range(act_thr):
                nc.scalar.activation(
                    out=junk_a[:],
                    in_=ids_all,
                    func=Sign,
                    bias=thr[:, j : j + 1],
                    scale=1.0,
                    accum_out=sgns[:, j : j + 1],
                )

            # ckey fields for two offsets at a time
            ck4 = big_pool.tile([128, 4 * NBLK, QCOL], i16, tag="ck4")

            for q in range(NQ):
                c0 = q * QCOL  # strip covers padded cols [c0, c0+1026)
                sl = slice(c0, c0 + QCOL + 2)

                typ = io_pool.tile([128, NBLK, QCOL + 2], i16, tag="typ")
                idn = io_pool.tile([128, NBLK, QCOL + 2], i16, tag="idn")
                tdn = io_pool.tile([128, NBLK, QCOL + 2], i16, tag="tdn")
                t3 = io_pool.tile([128, NBLK, QCOL + 2], i16, tag="t3")

                nc.sync.dma_start(out=typ[:], in_=typ_top[:, :, sl])
                # row-below tiles built on-chip: partition shift within SBUF
                fsl = slice(c0 + 1, c0 + 1 + QCOL + 2)  # same strip in idsF cols
                nc.sync.dma_start(out=idn[0:127, :, :], in_=idsF[1:128, :, fsl])
                nc.sync.dma_start(
                    out=idn[127:128, 0 : NBLK - 1, :], in_=idsF[0:1, 1:NBLK, fsl]
                )
                nc.sync.dma_start(
                    out=idn[127:128, NBLK - 1, :], in_=ids_d[ROWS : ROWS + 1, sl]
                )
                nc.sync.dma_start(out=tdn[0:127, :, :], in_=typ[1:128, :, :])
                nc.sync.dma_start(
                    out=tdn[127:128, 0 : NBLK - 1, :], in_=typ[0:1, 1:NBLK, :]
                )
                nc.sync.dma_start(
                    out=tdn[127:128, NBLK - 1, :], in_=typ_d[ROWS : ROWS + 1, sl]
                )

                # t3 = 3*typ + 1 on the ACT engine (the +1 lets the mask
                # fold multiplicatively: ck = (3t+tn+1)*[id!=idn] in {0,1..9})
                nc.scalar.activation(
                    t3[:], typ[:], mybir.ActivationFunctionType.Identity,
                    bias=ones[:, 0:1], scale=3.0,
                )

                # self views (payload cols of this quarter)
                ids_s = idsF[:, :, 2 + c0 : 2 + c0 + QCOL]
                t3_s = t3[:, :, 1 : QCOL + 1]

                # --- interaction ck fields: ck = (3t+tn+1)*[id!=idn] ---
                for o, (di, dj) in enumerate(OFFSETS):
                    if di == 0:
                        ids_n = idsF[:, :, 2 + c0 + dj : 2 + c0 + dj + QCOL]
                        t_n = typ[:, :, 1 + dj : QCOL + 1 + dj]
                    else:
                        ids_n = idn[:, :, 1 + dj : QCOL + 1 + dj]
                        t_n = tdn[:, :, 1 + dj : QCOL + 1 + dj]

                    s_ne = s_pool.tile([128, NBLK, QCOL], i16, tag="s_ne")
                    s_ky = s_pool.tile([128, NBLK, QCOL], i16, tag="dscratch")

                    nc.vector.tensor_tensor(
                        out=s_ne[:], in0=ids_s, in1=ids_n, op=A.not_equal
                    )
                    nc.vector.tensor_tensor(
                        out=s_ky[:], in0=t3_s, in1=t_n, op=A.add
                    )
                    nc.vector.tensor_tensor(
                        out=ck4[:, o * NBLK : (o + 1) * NBLK, :],
                        in0=s_ky[:],
                        in1=s_ne[:],
                        op=A.mult,
                    )
                # count 9 pair bins over all 4 offsets at once (bins 1..9)
                junk_c = s_pool.tile([128, 4 * NBLK, QCOL], i16, tag="dscratch")
                for v in range(NPAIR):
                    col = q * NPAIR + v
                    nc.vector.tensor_scalar(
                        out=junk_c[:],
                        in0=ck4[:],
                        scalar1=float(v + 1),
                        scalar2=None,
                        op0=A.is_equal,
                        op1=A.add,
                        accum_out=icnts[:, col : col + 1],
                    )

            # --- reduce partials across partitions with PE ones-matmul ---
            def pe_reduce(src, dst_dram, width):
                sb = acc_pool.tile([1, width], f32, tag=f"sb_{dst_dram.name}")
                for lo in range(0, width, 400):
                    hi = min(lo + 400, width)
                    ps = psum_pool.tile(
                        [1, 400], f32, tag=f"ps_{dst_dram.name}_{lo}", space="PSUM"
                    )
                    nc.tensor.matmul(
                        ps[:, : hi - lo], ones[:], src[:, lo:hi], start=True, stop=True
                    )
                    nc.scalar.copy(sb[:, lo:hi], ps[:, : hi - lo])
                nc.sync.dma_start(out=dst_dram[:, :], in_=sb[:])

            pe_reduce(counts, hist_d, dve_bins)
            pe_reduce(sgns, sgn_d, max(act_thr, 1))

            icnt_sum = acc_pool.tile([128, NPAIR], f32, tag="icnt_sum")
            # fold the NQ*2 groups: view [128, NQ*2, NPAIR] -> reduce groups on DVE
            nc.vector.tensor_reduce(
                out=icnt_sum[:],
                in_=icnts[:].rearrange("p (g v) -> p v g", v=NPAIR),
                op=A.add,
                axis=mybir.AxisListType.X,
            )
            pe_reduce(icnt_sum, icnt_d, NPAIR)

    nc.finalize()
    return nc


def _get_nc():
    if "nc" not in _CACHE:
        _CACHE["nc"] = _build()
    return _CACHE["nc"]


def _softplus(x):
    x = np.asarray(x, np.float64)
    return np.log1p(np.exp(-np.abs(x))) + np.maximum(x, 0.0)


def _make_in_maps(cell_ids, cell_types, dve_bins=DVE_BINS):
    ids = np.ascontiguousarray(cell_ids, dtype=np.int16)
    typ = np.ascontiguousarray(cell_types, dtype=np.int16)
    act_thr = NBINS - 1 - dve_bins
    if act_thr:
        thr = (0.5 - np.arange(dve_bins + 1, NBINS, dtype=np.float64)).astype(np.float32)
        thr = np.ascontiguousarray(thr.reshape(1, -1))
    else:
        thr = np.zeros((1, 1), np.float32)

    def shard(x, m):
        rows = np.arange(m * ROWS, m * ROWS + ROWS + 1) % H
        s = x[rows]  # [513, 4096]
        return np.ascontiguousarray(
            np.concatenate([s[:, -1:], s, s[:, :1]], axis=1)
        )  # [513, 4098]

    return [
        {"ids": shard(ids, m), "typ": shard(typ, m), "thr": thr}
        for m in range(NCORES)
    ]


def kernel(
    cell_ids, cell_types, J, gamma_J, bias_J, v_pref, lamb, offset, offset_scale
):
    nc = _get_nc()
    in_maps = _make_in_maps(cell_ids, cell_types)
    res = run_bass_kernel_spmd(nc, in_maps, core_ids=list(range(NCORES)))

    act_thr = NBINS - 1 - DVE_BINS
    hist = np.zeros(NBINS, np.float64)
    pair = np.zeros(NPAIR, np.float64)
    qpix = float(128 * NBLK * QCOL)  # pixels per quarter
    for r in res.results:
        hist[1 : DVE_BINS + 1] += r["hist_out"].reshape(DVE_BINS).astype(np.float64)
        if act_thr:
            S = r["sgn_out"].reshape(act_thr).astype(np.float64)  # S(b0+1..199)
            Sn = np.concatenate([S, [-4.0 * qpix]])  # append S(200)
            hist[DVE_BINS + 1 :] += (Sn[:-1] - Sn[1:]) / 2.0
        pair += r["icnt_out"].reshape(NPAIR).astype(np.float64)

    # symmetrize: ckey used 3*t_self + t_nbr with J symmetric
    J_eff = (
        _softplus(np.float64(gamma_J[0])) * np.asarray(J, np.float64)
        + np.float64(bias_J[0])
    )
    inter = float((J_eff.reshape(-1) * pair).sum()) / len(OFFSETS)
    vol = float(
        ((hist[1:] - np.float64(v_pref[0])) ** 2).sum()
        * (_softplus(np.float64(lamb[0])) + 0.001)
    )
    ham = vol + inter + float(offset[0]) * float(offset_scale[0])
    return np.array([ham], dtype=np.float32)
